# revision 16
# baseline (speedup 1.0000x reference)
"""Entropy-loss kernel for Trainium2, SPMD over 8 NeuronCores.

Reference computation (jax, f32):
    n_j   = sqrt(sum_i x_ij^2)              # column L2 norms (dim=0)
    p     = x / max(n_j, 1e-12)
    out   = mean_i( -sum_j p_ij * log(p_ij + 1e-8) )    # scalar

Sharding: columns (dim 1) split across 8 cores -> each core owns a
contiguous [R, 128] f32 shard (column-local normalization).

Math used by the kernel (single pass over HBM):
    with M_j = max(n_j, 1e-12),
      sum_ij p*log(p + 1e-8) = sum_j (1/M_j) * (A_j - log(M_j) * B_j)
      A_j = sum_i x_ij * log(x_ij + 1e-8 * M_j)
      B_j = sum_i x_ij
      C_j = sum_i x_ij^2          (n_j = sqrt(C_j))
    The 1e-8*M_j inside the log is replaced by the constant
    DELTA = 1e-8*sqrt(R/3) (the tight concentration value of n_j for
    uniform[0,1) fill); the substitution only matters for x < ~1e-5 where
    x*log(x+delta) < 1e-7, i.e. final rel err ~1e-12.

Gram-diagonal structure (all three column sums come from PE alone --
no DVE elementwise work at all):
    Split each chunk into BLOCKS of 128 free positions (one row per
    partition x 128 columns).  Per block r:
      ldweights W = x-block [128p, 128c]            (FWL, ~32cyc)
      mmC: moving = x-block,  FD=128 -> psC [128,128]  diag = sum x^2 (C)
      mmA: moving = ab-block, FD=128 -> psA [128,128]  diag = sum x*ln (A)
      mmB: moving = ones,     FD=1   -> psB [128,1]    = sum x (B)
    mmA/mmB reuse the W loaded by mmC (ins.ldweights=False).  Per block:
    1 ldweights + 3 matmuls ~= 125 ns warm -> ~64us PE total, under the
    ~81us DMA stream with catch-up slack.  The A/B pass for chunk j runs
    while chunk j+1's C pass streams, so PE never waits on ACT (ab).

    The PE HAM un-throttles (1.2 -> 2.4 GHz) only after ~3.4us of
    CONTINUOUS PE activity; the warm-up block is back-to-back matmuls
    (no interleaved ldweights -- those halve the duty cycle and the
    window never fills) sized to cross the threshold before chunk 0.

Per-core outputs: psC -> out_c [128,128]; psA|psB -> out_ab [128,129].
Host epilogue: C=diag(out_c), A=diag(out_ab[:,:128]), B=out_ab[:,128]
per core (columns are exact, no fold), then n=sqrt(C), combine, mean.
"""

import os

import numpy as np

import concourse.bass as bass
import concourse.tile as tile
from concourse import bacc, mybir
from concourse.bass_utils import run_bass_kernel_spmd

# Problem shape (fixed by the task).
R = 65536  # rows
C_TOTAL = 1024  # total columns
N_CORES = 8
C = C_TOTAL // N_CORES  # 128 columns per core

DELTA = 1e-8 * float(np.sqrt(R / 3.0))  # ~1.478e-6

F32 = mybir.dt.float32
BF16 = mybir.dt.bfloat16




def _chunk_schedule(rows_per_part: int, big: int = 32):
    """Row counts (per partition) per chunk: ramp-up, big chunks, tapered tail."""
    ramp = [4, 8, 16]
    taper = [16, 8, 4, 4]
    while sum(ramp) + sum(taper) > rows_per_part:
        ramp = ramp[1:]
        taper = taper[1:]
    n_big = (rows_per_part - sum(ramp) - sum(taper)) // big
    rem = rows_per_part - sum(ramp) - n_big * big - sum(taper)
    assert rem % 4 == 0
    sched = ramp + [big] * n_big + ([rem] if rem else []) + taper
    assert sum(sched) == rows_per_part
    return sched


def build_nc(
    rows: int = R,
    chunk_g: int = 32,
    n_warmup: int = 20,
    warmup_fd: int = 256,
    xb_bufs: int = 9,
):
    """Build the single-core Bass program for a [rows, 128] f32 shard."""
    assert rows % 128 == 0
    rows_per_part = rows // 128
    sched = _chunk_schedule(rows_per_part, big=chunk_g)

    nc = bacc.Bacc("TRN2", target_bir_lowering=False, debug=False)

    x = nc.dram_tensor("x", [rows, C], F32, kind="ExternalInput").ap()
    out_c = nc.dram_tensor("out_c", [C, C], F32, kind="ExternalOutput").ap()
    out_ab = nc.dram_tensor("out_ab", [C, C + 1], F32, kind="ExternalOutput").ap()

    # Contiguous-span partitioning: partition p owns rows
    # [p*rows/128, (p+1)*rows/128); chunk j covers sched[j] of those rows per
    # partition, read CONTIGUOUS per partition by the cast-DMA.
    xflat = x.rearrange("(p r) c -> p (r c)", p=128)

    with tile.TileContext(nc) as tc:
        with (
            tc.tile_pool(name="const", bufs=1) as const_pool,
            tc.tile_pool(name="xb", bufs=xb_bufs) as xb_pool,
            tc.tile_pool(name="ab", bufs=4) as ab_pool,
            tc.tile_pool(name="outp", bufs=1) as out_pool,
            tc.tile_pool(name="psum", bufs=1, space="PSUM") as psum_pool,
        ):
            # Constants on DVE (gpsimd runs ONLY dma_starts; DVE is idle).
            ones = const_pool.tile([128, 1], BF16)
            nc.vector.memset(ones, 1.0)
            delta_ap = const_pool.tile([128, 1], F32)
            nc.vector.memset(delta_ap, DELTA)
            warm = const_pool.tile([128, warmup_fd], BF16)
            nc.vector.memset(warm, 0.0)

            psC = psum_pool.tile([C, C], F32, tag="psC")
            psA = psum_pool.tile([C, C], F32, tag="psA")
            psB = psum_pool.tile([C, 1], F32, tag="psB")
            wacc = psum_pool.tile([1, warmup_fd], F32, tag="wacc")

            # PE warm-up: continuous back-to-back matmul activity crossing
            # the ~3.4us HAM window (ldweights suppressed after the first).
            for i in range(n_warmup):
                mi = nc.tensor.matmul(
                    wacc[:, :warmup_fd], ones[:, :], warm[:, :warmup_fd],
                    start=True, stop=True,
                )
                if i > 0:
                    mi.ins.ldweights = False

            big_free = max(sched) * C

            def c_pass(xb3, g, first, last=False):
                for r in range(g):
                    w = xb3[:, r, :]
                    nc.tensor.matmul(
                        psC, w, w,
                        start=(first and r == 0),
                        stop=(last and r == g - 1),
                    )

            def ab_pass(xb3, ab3, g, first, last=False):
                for r in range(g):
                    w = xb3[:, r, :]
                    st = first and r == 0
                    sp = last and r == g - 1
                    # This pass runs a chunk after c_pass reloaded other
                    # stationaries, so mmA must load W itself; mmB reuses it.
                    nc.tensor.matmul(
                        psA, w, ab3[:, r, :], start=st, stop=sp,
                    )
                    mi = nc.tensor.matmul(
                        psB, w, ones[:, 0:1], start=st, stop=sp,
                    )
                    mi.ins.ldweights = False

            row_off = 0
            prev = None  # (xb3, ab3, g, first) of the previous chunk
            for j, g in enumerate(sched):
                free = g * C
                xb = xb_pool.tile([128, big_free], BF16, tag="xb")
                nc.gpsimd.dma_start(
                    out=xb[:, :free],
                    in_=xflat[:, row_off * C : (row_off + g) * C],
                )
                xb3 = xb[:, :free].rearrange("p (g c) -> p g c", c=C)

                ab = ab_pool.tile([128, big_free], BF16, tag="ab")
                ab3 = ab[:, :free].rearrange("p (g c) -> p g c", c=C)
                nc.scalar.activation(
                    out=ab[:, :free],
                    in_=xb[:, :free],
                    func=mybir.ActivationFunctionType.Ln,
                    bias=delta_ap[:, :],
                    scale=1.0,
                )

                # C pass for THIS chunk (depends only on its DMA), then the
                # AB pass for the PREVIOUS chunk (its ab finished during this
                # chunk's stream) -- PE never waits on ACT.
                c_pass(xb3, g, j == 0, last=(j == len(sched) - 1))
                if prev is not None:
                    ab_pass(*prev)
                prev = (xb3, ab3, g, j == 0)
                row_off += g

            # Final AB pass closes its accumulation group.
            ab_pass(*prev, last=True)

            # psC is final after the last c_pass; copy on DVE while ACT
            # handles psA|psB.
            res_c = out_pool.tile([C, C], F32)
            nc.vector.tensor_copy(res_c, psC)
            nc.sync.dma_start(out=out_c, in_=res_c)

            res_ab = out_pool.tile([C, C + 1], F32)
            nc.scalar.copy(res_ab[:, 0:C], psA)
            nc.scalar.copy(res_ab[:, C : C + 1], psB)
            nc.sync.dma_start(out=out_ab, in_=res_ab)

    nc.compile()
    return nc


def host_epilogue(outs_c, outs_ab, rows: int) -> np.ndarray:
    """Combine per-core Gram outputs into the scalar loss."""
    total = 0.0
    for oc, oab in zip(outs_c, outs_ab):
        c = np.diag(oc.astype(np.float64))
        a = np.diag(oab[:, :C].astype(np.float64))
        b = oab[:, C].astype(np.float64)
        n = np.sqrt(np.maximum(c, 0.0))
        m_ = np.maximum(n, 1e-12)
        total += np.sum((a - np.log(m_) * b) / m_)
    return np.array(-total / rows, dtype=np.float32)


_NC_CACHE = {}


def kernel(target_prob: np.ndarray) -> np.ndarray:
    assert target_prob.shape == (R, C_TOTAL), target_prob.shape
    x = np.ascontiguousarray(target_prob, dtype=np.float32)

    key = "full"
    if key not in _NC_CACHE:
        _NC_CACHE[key] = build_nc()
    nc = _NC_CACHE[key]

    in_maps = [
        {"x": np.ascontiguousarray(x[:, c * C : (c + 1) * C])} for c in range(N_CORES)
    ]
    try:
        res = run_bass_kernel_spmd(nc, in_maps, core_ids=list(range(N_CORES)))
        outs_c = [r["out_c"] for r in res.results]
        outs_ab = [r["out_ab"] for r in res.results]
    except Exception:
        outs_c, outs_ab = _run_in_subprocess(x)
    return host_epilogue(outs_c, outs_ab, rows=R)


def _run_in_subprocess(x: np.ndarray):
    import subprocess
    import sys
    import tempfile

    with tempfile.TemporaryDirectory() as td:
        xp = os.path.join(td, "x.npy")
        op_c = os.path.join(td, "outs_c.npy")
        op_ab = os.path.join(td, "outs_ab.npy")
        np.save(xp, x)
        code = (
            "import sys, numpy as np\n"
            f"sys.path.insert(0, {os.path.dirname(os.path.abspath(__file__))!r})\n"
            "import kernel as K\n"
            f"x = np.load({xp!r})\n"
            "from concourse.bass_utils import run_bass_kernel_spmd\n"
            "nc = K.build_nc()\n"
            "in_maps = [{'x': np.ascontiguousarray(x[:, c*K.C:(c+1)*K.C])}"
            " for c in range(K.N_CORES)]\n"
            "res = run_bass_kernel_spmd(nc, in_maps, core_ids=list(range(K.N_CORES)))\n"
            f"np.save({op_c!r}, np.stack([r['out_c'] for r in res.results]))\n"
            f"np.save({op_ab!r}, np.stack([r['out_ab'] for r in res.results]))\n"
        )
        subprocess.run([sys.executable, "-c", code], check=True, timeout=1800)
        return list(np.load(op_c)), list(np.load(op_ab))
